# revision 80
# baseline (speedup 1.0000x reference)
"""DenseMPNN Trainium2 kernel (8-core SPMD, batch data-parallel).

Strategy (36.9us -> 22.4us vs the v1 baseline, TimelineSim):
- Shard batch B=32 across 8 cores (4 molecules/core); replicate weights.
- Host packs each molecule's ~4%-dense adjacency into an UNDIRECTED edge
  list (E_u <= 128): partition row e holds both directions of undirected
  edge {v,w} (fwd = v->w, bwd = w->v).
    H0[e,d] = relu(X[:,d,e]^T @ Wi)          X = [atoms[src]; bonds]
    iter:  HWh_d = H_d @ Wh                  (PE transpose + matmul)
           Q_d  = M_d0@HWh_0 + M_d1@HWh_1 + I@H0_d
           H_d  = relu(Q_d)
    out = relu(atoms@Wo_a + agg_final@Wo_h + bo)
  where M_de = inv_d (.) (G1_d @ T_e^T) - [e==1-d] diag(inv_d) are
  host-built [E,E] edge->edge message matrices: they fold the node
  aggregation (T), the source gather (G1), the reverse-edge subtraction
  and the 1/n_nbr scaling into ONE stationary operand, so the whole
  per-iteration update is matmuls + one PSUM->SBUF relu. (This removed
  the [N,H] P round trip and the DVE-only scalar_tensor_tensor of
  earlier versions: Pool cannot access PSUM, Act has no stt, so those
  ops pinned the critical engine.)
- bf16 operands (f32 PSUM accumulate): full-rate PE rows at any output
  width (f32r pays 4x below 256), 1.0 cyc/row transposes, DVE 2x/4x
  modes, half the DMA bytes. fp8 was tried and rejected: quantizing any
  matmul input costs ~3-4% on that term's output (errors do not average
  down against a random-sign-sum signal), which either fails the 2e-2
  gate (readout terms) or shifts the bottleneck to the PSUM-copy
  engines (H@Wh).
- Single-shot latency schedule: ~50 tiny warmup matmuls ramp the PE
  clock (1.54->0.42 ns/cycle over ~3us) during the DMA-in phase; a
  dependency-free dummy activation pulls the 1.3us relu-table load into
  the same window; inputs ride 5 large pages ordered by first use (one
  HWDGE costs ~625ns serialization + 650ns issue + 900ns completion
  semaphore, so few/large/early wins) with Wi on the parallel
  Pool/SWDGE path; emission is software-pipelined per molecule
  (engines run their queues in order, so each consumer is emitted
  right after its producer, interleaved across molecules); PSUM->SBUF
  copies are split between DVE (2x bf16 modes) and Act; the bf16
  output is stored in two halves so the store of molecules 0/1
  overlaps the readout of 2/3.
"""

import numpy as np

_B, _N, _A, _EB, _H = 32, 64, 133, 14, 256
_DEPTH = 3
_NCORES = 8
_MPC = _B // _NCORES  # molecules per core
_KX = _A + _EB  # 147

_cache = {}
_NWARM = 55  # PE clock-ramp warmup matmuls
_FP8_HWH = False  # H@Wh in fp8e4m3 DoubleRow (2x PE rate, one matmul per dir)


def _build_nc(E_u, reps=1):
    import sys
    for p in ("/opt/trn_rl_repo",):
        if p not in sys.path:
            sys.path.insert(0, p)
    import concourse.bass as bass  # noqa: F401
    import concourse.mybir as mybir
    import concourse.tile as tile
    from concourse import bacc
    from concourse.masks import make_identity

    BF = mybir.dt.bfloat16
    F8 = mybir.dt.float8e4
    F32 = mybir.dt.float32
    HT_N = _H // 128  # hidden chunks of 128
    RELU = mybir.ActivationFunctionType.Relu
    DR = mybir.MatmulPerfMode.DoubleRow

    E2 = 2 * E_u
    XC = 2 * E2 + 2                 # X1 | X2 | inv  columns per molecule
    GC = 64 + 64 + 2 * _N + 4 * E_u  # aT1 | aT2 | tm | M  columns per molecule

    nc = bacc.Bacc(None, target_bir_lowering=False, debug=False)

    # --- DRAM I/O (bf16 pages, ordered by first use) ---
    mx_d = nc.dram_tensor("mx", [_MPC, 128, XC], BF, kind="ExternalInput")
    wi_d = nc.dram_tensor("wi", [128, 512], BF, kind="ExternalInput")
    if _FP8_HWH:
        wh_d = nc.dram_tensor("wh", [128, 512], F8, kind="ExternalInput")
    else:
        wh_d = nc.dram_tensor("wh", [128, 512], BF, kind="ExternalInput")
    mg_d = nc.dram_tensor("mg", [_MPC, 128, GC], BF, kind="ExternalInput")
    wo_d = nc.dram_tensor("wo", [128, 1024], BF, kind="ExternalInput")
    out_d = nc.dram_tensor("out", [_MPC, _N, _H], BF, kind="ExternalOutput")

    with tile.TileContext(nc) as tc:
        import contextlib
        with contextlib.ExitStack() as ctx:
            consts = ctx.enter_context(tc.tile_pool(name="consts", bufs=1))
            work = ctx.enter_context(tc.tile_pool(name="work", bufs=8))
            hbuf = ctx.enter_context(tc.tile_pool(name="hbuf", bufs=8))
            ps_mm = ctx.enter_context(tc.tile_pool(name="ps_mm", bufs=4, space="PSUM"))
            ps_tr = ctx.enter_context(tc.tile_pool(name="ps_tr", bufs=2, space="PSUM"))

            # ---- PE warmup (independent of all loads): ramp the PE clock
            # from 0.65 GHz to 2.4 GHz while the DMAs fly. ----
            warm = consts.tile([128, 64], BF)
            nc.gpsimd.memset(warm, 0.0)
            # dependency-free first activation: pulls the 1.3us relu-table
            # load (LoadActFuncSet) into the DMA wait instead of mid-kernel
            warm_act = consts.tile([128, 64], BF)
            nc.scalar.activation(out=warm_act, in_=warm, func=RELU)
            ps_w = ps_tr.tile([64, 64], F32, tag="tr", name="ps_w")
            for i in range(_NWARM):
                nc.tensor.matmul(ps_w, warm, warm[:, 0:64], start=True, stop=True)

            # ---- loads: first molecules via SP/HWDGE, Wi on the parallel
            # Pool/SWDGE issue path, remaining pages in first-use order ----
            # molecule pages on SP/HWDGE (2 mols per page: HWDGE slots pace
            # one DMA per 625ns, so fewer/larger early pages win); Wi rides
            # the parallel Pool/SWDGE issue path.
            wi_s = consts.tile([128, 512], BF)
            nc.gpsimd.dma_start(out=wi_s, in_=wi_d[:, :])
            mx_s = consts.tile([128, _MPC, XC], BF)
            nc.sync.dma_start(out=mx_s[:, 0:2, :],
                              in_=mx_d[0:2].rearrange("m p c -> p m c"))
            nc.sync.dma_start(out=mx_s[:, 2:4, :],
                              in_=mx_d[2:4].rearrange("m p c -> p m c"))
            wh_s = consts.tile([128, HT_N, 256], F8 if _FP8_HWH else BF)
            nc.sync.dma_start(out=wh_s, in_=wh_d.rearrange("p (c n) -> p c n", c=HT_N))
            mg_s = consts.tile([128, _MPC, GC], BF)
            nc.sync.dma_start(out=mg_s, in_=mg_d.rearrange("m p c -> p m c"))
            wo_s = consts.tile([128, 1024], BF)
            nc.sync.dma_start(out=wo_s, in_=wo_d[:, :])

            # ---- small consts (Pool, after its SWDGE issue) ----
            ident = consts.tile([128, 128], BF)
            make_identity(nc, ident)

            wi1 = wi_s[:, 0:256]
            wi2 = wi_s[0:_KX - 128, 256:512]
            woa1 = wo_s[:, 0:256]
            woa2 = wo_s[0:_A + 1 - 128, 256:512]

            def mslice(m):
                s = {}
                s["X1"] = mx_s[:, m, 0:E2].rearrange("p (d e) -> p d e", d=2)
                s["X2"] = mx_s[0:_KX - 128, m, E2:2 * E2].rearrange(
                    "p (d e) -> p d e", d=2)
                s["aT1"] = mg_s[:, m, 0:64]
                s["aT2"] = mg_s[0:_A + 1 - 128, m, 64:128]
                s["tm"] = mg_s[0:E_u, m, 128:128 + 2 * _N].rearrange(
                    "p (d n) -> p d n", d=2)
                s["M"] = mg_s[0:E_u, m, 128 + 2 * _N:GC].rearrange(
                    "p (j e) -> p j e", j=4)  # j = 2*d + e
                return s

            def vrelu(eng, out, in_):
                if eng == 0:
                    nc.vector.tensor_scalar_max(out=out, in0=in_, scalar1=0.0)
                else:
                    nc.scalar.activation(out=out, in_=in_, func=RELU)

            for rep in range(reps):
                S = [mslice(m) for m in range(_MPC)]

                # ---- per-molecule emission helpers (software pipelining:
                # engines execute their queues IN ORDER, so consumers are
                # emitted immediately after producers, interleaved across
                # molecules, to avoid cross-phase stalls) ----
                def e_H0(m):
                    ps_h0 = ps_mm.tile([E_u, 2, _H], F32, tag="mm", name=f"psh0{m}")
                    for d in range(2):
                        nc.tensor.matmul(ps_h0[:, d, :], S[m]["X1"][:, d, :], wi1,
                                         start=True, stop=False)
                        nc.tensor.matmul(ps_h0[:, d, :], S[m]["X2"][:, d, :], wi2,
                                         start=False, stop=True)
                    S[m]["ps_h0"] = ps_h0

                def e_h0relu(m):
                    h0 = hbuf.tile([E_u, 2, _H], BF, tag="h0", name=f"h0_{m}")
                    # halves on both engines: halves the per-molecule latency
                    vrelu(0, h0[:, 0, :], S[m]["ps_h0"][:, 0, :])
                    vrelu(1, h0[:, 1, :], S[m]["ps_h0"][:, 1, :])
                    S[m]["h0"] = h0
                    S[m]["h"] = h0

                def e_T(m):
                    ps_t = ps_tr.tile([128, HT_N, 2, E_u], BF, tag="tr",
                                      name=f"pst{m}")
                    h = S[m]["h"]
                    for hh in range(HT_N):
                        for d in range(2):
                            nc.tensor.transpose(
                                ps_t[:, hh, d, :],
                                h[:, d, hh * 128:(hh + 1) * 128],
                                ident[:E_u, :E_u])
                    S[m]["ps_t"] = ps_t

                def e_ht(m):
                    ht = work.tile([128, HT_N, 2, E_u], F8 if _FP8_HWH else BF,
                                   tag="ht", name=f"ht{m}")
                    nc.vector.tensor_copy(out=ht, in_=S[m]["ps_t"])
                    S[m]["ht"] = ht

                def e_HWh(m):
                    ps_hw = ps_mm.tile([E_u, 2, _H], F32, tag="mm", name=f"pshw{m}")
                    for d in range(2):
                        if _FP8_HWH:
                            nc.tensor.matmul(ps_hw[:, d, :], S[m]["ht"][:, :, d, :],
                                             wh_s[:, :, :], start=True, stop=True,
                                             perf_mode=DR)
                        else:
                            for hh in range(HT_N):
                                nc.tensor.matmul(ps_hw[:, d, :],
                                                 S[m]["ht"][:, hh, d, :],
                                                 wh_s[:, hh, :], start=(hh == 0),
                                                 stop=(hh == HT_N - 1))
                    S[m]["ps_hw"] = ps_hw

                def e_hwh(m):
                    hwh = work.tile([E_u, 2, _H], BF, tag="hwh", name=f"hwh{m}")
                    nc.scalar.copy(out=hwh, in_=S[m]["ps_hw"])
                    S[m]["hwh"] = hwh

                def e_Q(m):
                    # Q_d = M_d0 @ HWh_0 + M_d1 @ HWh_1 + H0_d  (one PSUM group)
                    ps_q = ps_mm.tile([E_u, 2, _H], F32, tag="mm", name=f"psq{m}")
                    nc.tensor.matmul(ps_q, ident[:E_u, :E_u],
                                     S[m]["h0"].rearrange("e d h -> e (d h)"),
                                     start=True, stop=False, skip_group_check=True)
                    for d in range(2):
                        for e in range(2):
                            nc.tensor.matmul(ps_q[:, d, :],
                                             S[m]["M"][:, 2 * d + e, :],
                                             S[m]["hwh"][:, e, :], start=False,
                                             stop=(d == 1 and e == 1),
                                             skip_group_check=True)
                    S[m]["ps_q"] = ps_q

                def e_hn(m, eng):
                    hn = hbuf.tile([E_u, 2, _H], BF, tag="hn", name=f"hn{m}")
                    vrelu(eng, hn, S[m]["ps_q"])
                    S[m]["h"] = hn

                def e_A(m):
                    ps_a = ps_tr.tile([128, HT_N, _N], F32, tag="tr", name=f"psa{m}")
                    h = S[m]["h"]
                    for hh in range(HT_N):
                        for d in range(2):
                            nc.tensor.matmul(ps_a[:, hh, :],
                                             h[:, d, hh * 128:(hh + 1) * 128],
                                             S[m]["tm"][:, d, :],
                                             start=(d == 0), stop=(d == 1))
                    S[m]["ps_a"] = ps_a

                def e_af(m):
                    af = work.tile([128, HT_N, _N], BF, tag="af", name=f"af{m}")
                    nc.vector.tensor_copy(out=af, in_=S[m]["ps_a"])
                    S[m]["af"] = af

                def e_O(m):
                    ps_o = ps_mm.tile([_N, _H], F32, tag="mm", name=f"pso{m}")
                    nc.tensor.matmul(ps_o, S[m]["aT1"], woa1, start=True, stop=False)
                    nc.tensor.matmul(ps_o, S[m]["aT2"], woa2, start=False, stop=False)
                    for hh in range(HT_N):
                        nc.tensor.matmul(ps_o, S[m]["af"][:, hh, :],
                                         wo_s[:, 512 + hh * 256:512 + (hh + 1) * 256],
                                         start=False, stop=(hh == HT_N - 1))
                    S[m]["ps_o"] = ps_o

                # ---- entry: H0 with iter-0 transposes interleaved.
                # e_warm emits dependency-free filler matmuls so the in-order
                # PE queue doesn't idle while a transpose waits its relu. ----
                def e_warm(k):
                    for _ in range(k):
                        nc.tensor.matmul(ps_w, warm, warm[:, 0:64],
                                         start=True, stop=True)
                e_H0(0); e_H0(1)
                e_h0relu(0); e_T(0); e_ht(0)
                e_H0(2)
                e_h0relu(1); e_H0(3)
                e_h0relu(2)
                e_T(1); e_ht(1)
                e_T(2); e_ht(2)
                e_h0relu(3); e_T(3); e_ht(3)

                # ---- message passing iterations (next-phase transposes /
                # readout aggregations interleaved into the Q phase) ----
                for it in range(_DEPTH - 1):
                    last = (it == _DEPTH - 2)

                    def e_next(m):
                        if last:
                            e_A(m); e_af(m)
                        else:
                            e_T(m); e_ht(m)

                    for m in range(_MPC):
                        e_HWh(m); e_hwh(m)
                    e_Q(0); e_hn(0, 0)
                    e_Q(1)
                    e_Q(2); e_next(0)
                    e_hn(1, 0)
                    e_Q(3); e_hn(2, 1)
                    e_next(1)
                    e_hn(3, 1)
                    e_next(2); e_next(3)

                # ---- readout matmuls + relu + single merged store ----
                # bf16 stores in two halves; whole-mol relus alternate engines
                # so the last molecule's relu isn't queued behind the others
                o_all = consts.tile([_N, _MPC, _H], BF, name="o_all")
                e_O(0); vrelu(0, o_all[:, 0, :], S[0]["ps_o"])
                e_O(1); vrelu(1, o_all[:, 1, :], S[1]["ps_o"])
                nc.sync.dma_start(out=out_d[0:2].rearrange("m n h -> n m h"),
                                  in_=o_all[:, 0:2, :])
                e_O(2); vrelu(0, o_all[:, 2, :], S[2]["ps_o"])
                e_O(3); vrelu(1, o_all[:, 3, :], S[3]["ps_o"])
                nc.sync.dma_start(out=out_d[2:4].rearrange("m n h -> n m h"),
                                  in_=o_all[:, 2:4, :])

    nc.compile()
    return nc


def _prep_inputs(atoms, bonds, adj, Wi, Wh, Wo, bo):
    import ml_dtypes
    BF = np.dtype(ml_dtypes.bfloat16)
    B, N, A = atoms.shape
    H = Wh.shape[0]

    und = []
    for b in range(B):
        vw = np.argwhere(np.triu(adj[b]) > 0)  # canonical (v < w)
        und.append(vw)
    E_max = max(len(e) for e in und)
    E_u = max(32, ((E_max + 31) // 32) * 32)
    assert E_u <= 128, f"E_u={E_u} exceeds one partition tile"

    E2 = 2 * E_u
    XC = 2 * E2 + 2
    GC = 64 + 64 + 2 * N + 4 * E_u
    mx = np.zeros((B, 128, XC), np.float32)
    mg = np.zeros((B, 128, GC), np.float32)

    for b in range(B):
        vw = und[b]
        E = len(vw)
        v_e, w_e = vw[:, 0], vw[:, 1]
        deg = adj[b].sum(1)
        ar = np.arange(E)

        # X[:, d, e] = [atoms[src(e,d)] ; bonds(e,d)]  (KX = 133+14 rows)
        X = np.zeros((_KX, 2, E_u), np.float32)
        X[:A, 0, :E] = atoms[b, v_e].T
        X[:A, 1, :E] = atoms[b, w_e].T
        X[A:, 0, :E] = bonds[b, v_e, w_e].T
        X[A:, 1, :E] = bonds[b, w_e, v_e].T
        mx[b, :, 0:E2] = X[0:128].reshape(128, E2)
        mx[b, 0:_KX - 128, E2:2 * E2] = X[128:].reshape(_KX - 128, E2)
        inv = np.zeros((E_u, 2), np.float32)
        inv[:E, 0] = 1.0 / np.maximum(deg[v_e] - 1.0, 1.0)
        inv[:E, 1] = 1.0 / np.maximum(deg[w_e] - 1.0, 1.0)
        mx[b, 0:E_u, 2 * E2:2 * E2 + 2] = inv  # kept for reference/debug

        atomsT = np.zeros((A + 1, N), np.float32)
        atomsT[:A] = atoms[b].T
        atomsT[A] = 1.0
        src = np.zeros((2, E_u), np.int64)  # src node of edge (d, e)
        tgt = np.zeros((2, E_u), np.int64)  # tgt node of edge (d, e)
        src[0, :E], src[1, :E] = v_e, w_e
        tgt[0, :E], tgt[1, :E] = w_e, v_e
        Tfb = np.zeros((E_u, 2, N), np.float32)
        Tfb[ar, 0, w_e] = 1.0
        Tfb[ar, 1, v_e] = 1.0
        # M_de[e1,e2] = inv_d[e1] * [src_d(e1) == tgt_e(e2)]
        #   - [e == 1-d] inv_d[e1] * [e1 == e2]
        # stored transposed (lhsT layout): band[:, 2d+e, :][e2, e1] = M_de[e1, e2]
        Mband = np.zeros((E_u, 4, E_u), np.float32)
        for d in range(2):
            for e in range(2):
                Mde = (src[d][:, None] == tgt[e][None, :]).astype(np.float32)
                if E < E_u:
                    Mde[E:, :] = 0.0
                    Mde[:, E:] = 0.0
                Mde *= inv[:, d][:, None]
                if e == 1 - d:
                    Mde -= np.diag(inv[:, d])
                Mband[:, 2 * d + e, :] = Mde.T
        mg[b, 0:128, 0:64] = atomsT[0:128]
        mg[b, 0:A + 1 - 128, 64:128] = atomsT[128:]
        mg[b, 0:E_u, 128:128 + 2 * N] = Tfb.reshape(E_u, 2 * N)
        mg[b, 0:E_u, 128 + 2 * N:GC] = Mband.reshape(E_u, 4 * E_u)

    wi = np.zeros((128, 512), np.float32)
    wi[:, 0:256] = Wi[0:128]
    wi[0:_KX - 128, 256:512] = Wi[128:]
    wh = Wh.reshape(2, 128, 256).transpose(1, 0, 2).reshape(128, 512)
    wo = np.zeros((128, 1024), np.float32)
    wo[:, 0:256] = Wo[0:128]
    wo[0:A + 1 - 128, 256:512] = np.concatenate([Wo[128:A], bo[None, :]], axis=0)
    wo[:, 512:1024] = Wo[A:].reshape(2, 128, 256).transpose(1, 0, 2).reshape(128, 512)

    F8 = np.dtype(ml_dtypes.float8_e4m3)
    shared = {
        "wi": wi.astype(BF),
        "wh": np.ascontiguousarray(wh).astype(F8 if _FP8_HWH else BF),
        "wo": wo.astype(BF),
    }

    def shard(x):
        return x.reshape((_NCORES, _MPC) + x.shape[1:])

    mx8, mg8 = shard(mx.astype(BF)), shard(mg.astype(BF))
    per_core = [
        {"mx": mx8[c], "mg": mg8[c], **shared}
        for c in range(_NCORES)
    ]
    return per_core, E_u


def kernel(atoms, bonds, adj, Wi, Wh, Wo, bo, _trace=False):
    import sys
    for p in ("/opt/trn_rl_repo",):
        if p not in sys.path:
            sys.path.insert(0, p)
    from concourse.bass_utils import run_bass_kernel_spmd

    atoms = np.asarray(atoms, np.float32)
    bonds = np.asarray(bonds, np.float32)
    adj = np.asarray(adj, np.float32)
    Wi = np.asarray(Wi, np.float32)
    Wh = np.asarray(Wh, np.float32)
    Wo = np.asarray(Wo, np.float32)
    bo = np.asarray(bo, np.float32)

    in_maps, E_u = _prep_inputs(atoms, bonds, adj, Wi, Wh, Wo, bo)

    key = ("nc", E_u)
    if key not in _cache:
        _cache[key] = _build_nc(E_u)
    nc = _cache[key]

    res = run_bass_kernel_spmd(nc, in_maps, list(range(_NCORES)), trace=_trace)
    outs = [res.results[c]["out"] for c in range(_NCORES)]
    full = np.concatenate(outs, axis=0).reshape(_B, _N, _H).astype(np.float32)
    if _trace:
        return full, res
    return full
